# revision 1
# baseline (speedup 1.0000x reference)
"""DescrptSeA descriptor kernel for 8 Trainium2 NeuronCores.

Data-parallel sharding per the problem's sharding hint: the nloc (local atom)
axis is split into 8 equal shards, one per core. The neighbor-list gather is
performed host-side (the neuron compiler's codegen currently asserts on
indirect_load for this access pattern); each core then computes the smoothed
environment matrix, runs the 3-layer embedding net (1->25->50->100, tanh,
resnet-doubling skips) over its (atom, neighbor) points, and contracts to the
[M*AXIS] descriptor on device. Shard outputs are concatenated on the host into
the full [nf, nloc, 1600] float32 output.
"""

import numpy as np
import jax
import jax.numpy as jnp

NF, NLOC, NALL = 2, 4096, 8192
NTYPES = 2
SEL = [46, 92]
NNEI = sum(SEL)
SEC = [0, 46, 138]
NEURON = [25, 50, 100]
AXIS = 16
RCUT, RCUT_SMTH = 6.0, 0.5
PROT = 1e-6

NCORES = 8
SHARD = NLOC // NCORES  # 512 atoms per core


def _smooth_weight(d, rmin, rmax):
    uu = (d - rmin) / (rmax - rmin)
    uu = jnp.clip(uu, 0.0, 1.0)
    return uu * uu * uu * (-6.0 * uu * uu + 15.0 * uu - 10.0) + 1.0


def _shard_fn(coord_r, centers, mask, nscale, nshift,
              w0, b0, w1, b1, w2, b2):
    # coord_r [nf, shard, nnei, 3]; centers [nf, shard, 3]
    # mask [nf, shard, nnei] f32; nscale/nshift [nf, shard, nnei, 4] f32
    nf, nloc, nnei, _ = coord_r.shape
    diff = coord_r - centers[:, :, None, :]
    length = jnp.sqrt(jnp.sum(diff * diff, axis=-1, keepdims=True))
    m = mask[..., None]
    length = length * m + (1.0 - m)  # padding entries -> length 1.0
    t0 = 1.0 / (length + PROT)
    t1 = diff / ((length + PROT) ** 2)
    w = _smooth_weight(length, RCUT_SMTH, RCUT) * m
    env = jnp.concatenate([t0, t1], axis=-1) * w
    dm = env * nscale + nshift

    dm = dm.reshape(nf * nloc, nnei, 4)
    M = w2.shape[-1]
    xyz = jnp.zeros((nf * nloc, 4, M), dm.dtype)
    for t in range(NTYPES):
        rr = dm[:, SEC[t] : SEC[t + 1], :]
        x = rr[:, :, :1]
        for W, b in ((w0[t], b0[t]), (w1[t], b1[t]), (w2[t], b2[t])):
            y = jnp.tanh(x @ W + b)
            if W.shape[-1] == 2 * x.shape[-1]:
                y = y + jnp.concatenate([x, x], axis=-1)
            elif W.shape[-1] == x.shape[-1]:
                y = y + x
            x = y
        xyz = xyz + jnp.einsum("nsc,nsm->ncm", rr, x)
    xyz = xyz / NNEI
    res = jnp.einsum("ncm,nca->nma", xyz, xyz[:, :, :AXIS])
    return res.reshape(nf, nloc, M * AXIS)


_pmapped = None


def _get_pmapped():
    global _pmapped
    if _pmapped is None:
        _pmapped = jax.pmap(_shard_fn, devices=jax.devices()[:NCORES])
    return _pmapped


def kernel(nlist, extended_coord, extended_atype, mean, stddev,
           w0, b0, w1, b1, w2, b2):
    nlist = np.asarray(nlist)
    coord = np.asarray(extended_coord, dtype=np.float32)  # [nf, nall, 3]
    atype = np.asarray(extended_atype)

    mask = (nlist >= 0)
    nl = np.where(mask, nlist, 0).astype(np.int64)  # [nf, nloc, nnei]

    # host-side neighbor gather (index marshalling for the device kernel)
    fidx = np.arange(NF)[:, None, None]
    coord_r = coord[fidx, nl, :]  # [nf, nloc, nnei, 3]
    centers = coord[:, :NLOC, :]  # [nf, nloc, 3]

    # shard the nloc axis across the 8 cores
    def sh(x, extra):  # [nf, nloc, ...] -> [8, nf, shard, ...]
        return np.ascontiguousarray(
            x.reshape((NF, NCORES, SHARD) + extra).transpose((1, 0, 2) + tuple(
                3 + i for i in range(len(extra))))
        )

    coord_r_sh = sh(coord_r, (NNEI, 3)).astype(np.float32)
    centers_sh = sh(centers, (3,)).astype(np.float32)
    mask_sh = sh(mask.astype(np.float32), (NNEI,))
    atype_loc = atype[:, :NLOC].astype(np.int64)
    nscale = (1.0 / np.asarray(stddev, np.float32))[atype_loc]  # [nf,nloc,nnei,4]
    nshift = (-np.asarray(mean, np.float32) / np.asarray(stddev, np.float32))[atype_loc]
    nscale_sh = sh(nscale, (NNEI, 4)).astype(np.float32)
    nshift_sh = sh(nshift, (NNEI, 4)).astype(np.float32)

    def rep(x):
        x = np.asarray(x, dtype=np.float32)
        return np.broadcast_to(x, (NCORES,) + x.shape).copy()

    out = _get_pmapped()(
        coord_r_sh, centers_sh, mask_sh, nscale_sh, nshift_sh,
        rep(w0), rep(b0), rep(w1), rep(b1), rep(w2), rep(b2),
    )  # [8, nf, shard, M*AXIS]

    out = np.asarray(out)
    full = out.transpose(1, 0, 2, 3).reshape(NF, NLOC, NEURON[-1] * AXIS)
    return np.ascontiguousarray(full.astype(np.float32))

